# revision 23
# baseline (speedup 1.0000x reference)
"""Trainium2 Bass kernel for CachedRoPEAttention.

Sharding: 8 cores = batch(2) x head-groups(4). Each core computes 4 heads of
one batch element end-to-end (q/k/v proj, RoPE, causal attention with
ones-row softmax denominators, out_proj partial); host sums the 4
tensor-parallel partials per batch.

v2: bf16 end-to-end (fp32 PSUM accumulation). Halves HBM traffic and DVE
cycles vs the f32r baseline; matmul stream rate is unchanged but the
schedule is restructured to keep the PE dense (HAM stays warm):
 - projections: stationary weights, 4 MMs per LDWEIGHTS, N=512
 - attention: per-key-tile software pipeline (scores -> exp -> mask -> PV)
   with PSUM budget mm(2) + st(4) + ot(2) = 8 banks
 - out_proj for block b is emitted one attention block late so its matmuls
   fill PE gaps in the ACT-limited attention tail.
"""
import sys
sys.path.insert(0, "/opt/trn_rl_repo")

import numpy as np
import ml_dtypes

import concourse.bass as bass
import concourse.bacc as bacc
import concourse.mybir as mybir
import concourse.tile as tile
from concourse.bass_utils import run_bass_kernel_spmd

F32 = mybir.dt.float32
F32R = mybir.dt.float32r
BF16 = mybir.dt.bfloat16
NPBF16 = ml_dtypes.bfloat16

D, H, DH, T, B = 1024, 16, 64, 2048, 2
HG, HPC, EC = 4, 4, 256      # head groups, heads/core, e-width/core
KT = D // 128                # 8 contraction tiles over d_model
NB = T // 512                # 4 t-blocks
NTT = T // 128               # 16 t-tiles

_NC_CACHE = {}


def _build_nc():
    nc = bacc.Bacc(None, target_bir_lowering=False)

    # all inputs host-packed partition-major so every DMA is 128 descriptors
    # of >=4KB per-partition-contiguous data (HWDGE desc-gen is ~7ns/desc;
    # row-strided layouts explode into 512B descriptors and serialize the
    # load phase)
    xP_d = nc.dram_tensor("xP", [128, 2 * 2 * 4 * 1024], BF16,
                          kind="ExternalInput")   # [p][parity][half][k2][t]
    wqP_d = nc.dram_tensor("wqP", [128, KT * EC], BF16, kind="ExternalInput")
    wkP_d = nc.dram_tensor("wkP", [128, KT * EC], BF16, kind="ExternalInput")
    wvP_d = nc.dram_tensor("wvP", [128, KT * EC], BF16, kind="ExternalInput")
    woP_d = nc.dram_tensor("woP", [128, 2 * D], BF16, kind="ExternalInput")
    cos2_d = nc.dram_tensor("cos2", [128, T], BF16, kind="ExternalInput")
    sin2p_d = nc.dram_tensor("sin2p", [128, T], BF16, kind="ExternalInput")
    tri2_d = nc.dram_tensor("tri2", [128, 256], BF16, kind="ExternalInput")
    outT_d = nc.dram_tensor("outT", [D, T], BF16, kind="ExternalOutput")

    with tile.TileContext(nc) as tc:
        with tc.tile_pool(name="perm", bufs=1) as perm, \
             tc.tile_pool(name="psum", bufs=1, space="PSUM") as psp, \
             tc.tile_pool(name="rw", bufs=3) as rw, \
             tc.tile_pool(name="ew", bufs=3) as ew:
            # ---- persistent tiles; x parity-major so one strided DMA per
            # (parity, t-half) covers 4 k-tiles
            x_sb = perm.tile([128, 2, KT // 2, T], BF16)
            wq_sb = perm.tile([128, KT, EC], BF16)
            wk_sb = perm.tile([128, KT, EC], BF16)
            wv_sb = perm.tile([128, KT, EC], BF16)
            wo_sb = perm.tile([128, 2, D], BF16)
            cos_sb = perm.tile([128, T], BF16)
            sin_sb = perm.tile([128, T], BF16)
            tri_sb = perm.tile([128, 2, 128], BF16)
            qT = perm.tile([128, 2, T], BF16)
            kT = perm.tile([128, 2, T], BF16)
            v_sb = perm.tile([128, NTT, HPC, 65], BF16)
            OT_all = perm.tile([128, 2, T], BF16)
            ones_sb = perm.tile([1, 64], F32R)

            def xk(k):
                return x_sb[:, k % 2, k // 2, :]

            # DRAM views (per-partition contiguous on the inner dims)
            Xv = xP_d.ap().rearrange("p (a h b t) -> p a h b t",
                                     a=2, h=2, b=4)

            # ---- input DMAs in consumption order.
            # scalar queue: wv, tri, x(odd,h0), wq, x(odd,h1), wo
            # sync queue:   x(even,h0), cos, sin, wk, x(even,h1)
            nc.scalar.dma_start(out=wv_sb.rearrange("p a b -> p (a b)"),
                                in_=wvP_d.ap())
            nc.scalar.dma_start(out=tri_sb.rearrange("p a b -> p (a b)"),
                                in_=tri2_d.ap())
            nc.sync.dma_start(out=x_sb[:, 0, :, 0:1024], in_=Xv[:, 0, 0])
            nc.scalar.dma_start(out=x_sb[:, 1, :, 0:1024], in_=Xv[:, 1, 0])
            nc.sync.dma_start(out=cos_sb, in_=cos2_d.ap())
            nc.sync.dma_start(out=sin_sb, in_=sin2p_d.ap())
            nc.scalar.dma_start(out=wq_sb.rearrange("p a b -> p (a b)"),
                                in_=wqP_d.ap())
            nc.sync.dma_start(out=wk_sb.rearrange("p a b -> p (a b)"),
                              in_=wkP_d.ap())
            nc.sync.dma_start(out=x_sb[:, 0, :, 1024:2048], in_=Xv[:, 0, 1])
            nc.scalar.dma_start(out=x_sb[:, 1, :, 1024:2048], in_=Xv[:, 1, 1])
            nc.scalar.dma_start(out=wo_sb.rearrange("p a b -> p (a b)"),
                                in_=woP_d.ap())

            # ones: v denominator column (bf16) + f32r ones row for the
            # reciprocal-broadcast matmuls
            ones1 = rw.tile([128, 1], F32, tag="ones1", bufs=1)
            nc.vector.memset(ones1, 1.0)
            nc.vector.tensor_copy(
                out=v_sb[:, :, :, 64:65].rearrange("p a b c -> p (a b c)"),
                in_=ones1[:, 0:1].broadcast_to((128, NTT * HPC)))
            nc.vector.tensor_copy(out=ones_sb,
                                  in_=ones1[0:1, 0:1].broadcast_to((1, 64)))
            # warm the exp table load off the critical path
            warm = rw.tile([1, 1], BF16, tag="warm", bufs=1)
            nc.scalar.activation(out=warm, in_=ones1[0:1, 0:1],
                                 func=mybir.ActivationFunctionType.Exp)

            # ---------------- helpers ----------------
            def v_proj(tt_lo, tt_hi):
                # v[t, e] for t-tiles [tt_lo, tt_hi): stationary x tile,
                # moving wv (N=256)
                for tt in range(tt_lo, tt_hi):
                    ps = psp.tile([128, 512], F32, tag="mm", bufs=2,
                                  name=f"psv{tt}")
                    for k in range(KT):
                        nc.tensor.matmul(
                            ps[:, 0:256],
                            xk(k)[:, 128 * tt:128 * tt + 128],
                            wv_sb[:, k, :],
                            start=(k == 0), stop=(k == KT - 1))
                    # evacuate on ACT (idle through the whole projection
                    # phase) so the DVE queue that gates the RoPE chain --
                    # and with it the attention start -- stays short
                    nc.scalar.copy(
                        out=v_sb[:, tt, :, 0:64],
                        in_=ps[:, 0:256].rearrange("p (h d) -> p h d", h=HPC))

            def qk_proj_half(w_sb, dst, pt, half, wnm):
                # two 512-col t-blocks (nb = 2*half, 2*half+1) of q or k for
                # partition-tile pt: stationary w chunk, 2 MMs per LDW,
                # fp32 PSUM accumulate over k, then fused RoPE per block.
                ps = psp.tile([128, 2, 512], F32, tag="st", bufs=2,
                              name=f"ps{wnm}{pt}{half}")
                for k in range(KT):
                    for j in range(2):
                        cols = slice(512 * (2 * half + j),
                                     512 * (2 * half + j) + 512)
                        nc.tensor.matmul(
                            ps[:, j, :],
                            w_sb[:, k, 128 * pt:128 * pt + 128],
                            xk(k)[:, cols],
                            start=(k == 0), stop=(k == KT - 1))
                cols = slice(1024 * half, 1024 * half + 1024)
                psf = ps.rearrange("p a b -> p (a b)")
                qc = rw.tile([128, 1024], BF16, tag="qc")
                nc.vector.tensor_mul(out=qc, in0=psf, in1=cos_sb[:, cols])
                # sin term: multiply by pre-shuffled sin const, THEN
                # partition-shuffle (p ^ 32) the product via DMA
                qp = rw.tile([128, 1024], BF16, tag="qp")
                nc.vector.tensor_mul(out=qp, in0=psf, in1=sin_sb[:, cols])
                shuf = rw.tile([128, 1024], BF16, tag="shuf")
                for b2 in range(2):
                    base = 64 * b2
                    nc.sync.dma_start(out=shuf[base:base + 32, :],
                                      in_=qp[base + 32:base + 64, :])
                    nc.sync.dma_start(out=shuf[base + 32:base + 64, :],
                                      in_=qp[base:base + 32, :])
                nc.vector.tensor_add(out=dst[:, pt, cols],
                                     in0=qc, in1=shuf)

            def attention(blk, pair):
                # causal attention for q-block blk (512 queries), heads
                # 2*pair + {0,1}; per-key-tile pipeline.
                qcols = slice(512 * blk, 512 * blk + 512)
                nkt = 4 * (blk + 1)
                ot = psp.tile([65, 2, 512], F32, tag="ot", bufs=1,
                              name=f"ot{blk}{pair}")
                for kt in range(nkt):
                    lop = max(0, 128 * kt - 512 * blk)
                    st = psp.tile([128, 2, 512], F32, tag="st", bufs=2,
                                  name=f"st{blk}{pair}{kt}")
                    for hd in range(2):
                        hrow = slice(64 * hd, 64 * hd + 64)
                        nc.tensor.matmul(
                            st[:, hd, lop:512],
                            kT[hrow, pair, 128 * kt:128 * kt + 128],
                            qT[hrow, pair, 512 * blk + lop:512 * blk + 512],
                            start=True, stop=True)
                    ex = ew.tile([128, 2, 512], BF16, tag="ex", bufs=4,
                                 name=f"ex{blk}{pair}{kt}")
                    nc.scalar.activation(
                        out=ex[:, :, lop:512], in_=st[:, :, lop:512],
                        func=mybir.ActivationFunctionType.Exp, scale=0.125)
                    if kt >= 4 * blk:  # diagonal tile: causal mask (gpsimd
                        # keeps this off the DVE critical path)
                        nc.gpsimd.tensor_mul(
                            out=ex[:, :, lop:lop + 128],
                            in0=ex[:, :, lop:lop + 128],
                            in1=tri_sb)
                    for hd in range(2):
                        nc.tensor.matmul(
                            ot[:, hd, lop:512],
                            v_sb[:, kt, 2 * pair + hd, :],
                            ex[:, hd, lop:512],
                            start=(kt == 0), stop=(kt == nkt - 1))
                # softmax denominators: ones row (partition 64 of ot) ->
                # f32r copy -> PE broadcast to 64 partitions -> approx
                # reciprocal -> normalize muls into OT_all
                den = ew.tile([1, 2, 512], F32R, tag="den",
                              name=f"den{blk}{pair}")
                rc = ew.tile([64, 2, 512], F32, tag="rc",
                             name=f"rc{blk}{pair}")
                for hd in range(2):
                    # per-hd den copy so the first broadcast starts after
                    # 0.6us instead of 1.2us (frees ot banks sooner)
                    nc.vector.tensor_copy(out=den[0:1, hd, :],
                                          in_=ot[64:65, hd, :])
                    dbc = psp.tile([64, 512], F32, tag="mm", bufs=2,
                                   name=f"dbc{blk}{pair}{hd}")
                    nc.tensor.matmul(dbc, ones_sb, den[0:1, hd, :],
                                     start=True, stop=True)
                    nc.vector.reciprocal_approx_fast(
                        out=rc[:, hd, :], in_=dbc)
                for hd in range(2):
                    nc.vector.tensor_mul(
                        out=OT_all[64 * hd:64 * hd + 64, pair, qcols],
                        in0=ot[0:64, hd, :], in1=rc[:, hd, :])

            def out_proj(blk, mlo, mhi):
                cols = slice(512 * blk, 512 * blk + 512)
                for m in range(mlo, mhi):
                    fp = psp.tile([128, 512], F32, tag="mm", bufs=2,
                                  name=f"fp{blk}{m}")
                    for ct in range(2):
                        nc.tensor.matmul(
                            fp, wo_sb[:, ct, 128 * m:128 * m + 128],
                            OT_all[:, ct, cols],
                            start=(ct == 0), stop=(ct == 1))
                    fs = ew.tile([128, 512], BF16, tag="fs",
                                 name=f"fs{blk}{m}")
                    nc.vector.tensor_copy(out=fs, in_=fp)
                    nc.sync.dma_start(
                        out=outT_d.ap()[128 * m:128 * m + 128, cols], in_=fs)

            # ---------------- schedule ----------------
            v_proj(0, 4)
            qk_proj_half(wq_sb, qT, 0, 0, "q")
            qk_proj_half(wk_sb, kT, 0, 0, "k")
            attention(0, 0)
            qk_proj_half(wq_sb, qT, 0, 1, "q")
            qk_proj_half(wk_sb, kT, 0, 1, "k")
            v_proj(4, 8)
            qk_proj_half(wq_sb, qT, 1, 0, "q")
            qk_proj_half(wk_sb, kT, 1, 0, "k")
            attention(0, 1)
            qk_proj_half(wq_sb, qT, 1, 1, "q")
            qk_proj_half(wk_sb, kT, 1, 1, "k")
            v_proj(8, 12)
            attention(1, 0)
            v_proj(12, 14)
            attention(1, 1)
            out_proj(0, 0, 4)
            attention(2, 0)
            v_proj(14, 16)
            out_proj(0, 4, 8)
            attention(2, 1)
            out_proj(1, 0, 4)
            attention(3, 0)
            out_proj(1, 4, 8)
            out_proj(2, 0, 4)
            attention(3, 1)
            out_proj(2, 4, 8)
            out_proj(3, 0, 8)

    nc.compile()
    return nc


def _consts():
    i = np.arange(32)
    theta = 1.0 / (10000.0 ** (2.0 * i / 64))
    ang = np.outer(np.arange(T, dtype=np.float64), theta)
    p = np.arange(128)
    cos2 = np.cos(ang[:, p % 32]).T
    sgn = np.where((p % 64) < 32, -1.0, 1.0)
    sin2s = (np.sin(ang[:, p % 32]) * sgn).T
    # pre-shuffled sin so the kernel can multiply BEFORE the partition
    # shuffle: shuf(q * sin2p)[p] = q[p^32] * sin2s[p]
    sin2p = sin2s[p ^ 32]
    r, c = np.meshgrid(np.arange(128), np.arange(128), indexing="ij")
    tri = (r <= c).astype(np.float64)
    tri2 = np.broadcast_to(tri[:, None, :], (128, 2, 128)).reshape(128, 256)
    cos2 = np.ascontiguousarray(cos2).astype(NPBF16)
    sin2p = np.ascontiguousarray(sin2p).astype(NPBF16)
    tri2 = np.ascontiguousarray(tri2).astype(NPBF16)
    return cos2, sin2p, tri2


def kernel(x, Wq, Wk, Wv, Wo, _trace=False):
    x = np.asarray(x, dtype=np.float32)
    Wq = np.asarray(Wq, dtype=np.float32)
    Wk = np.asarray(Wk, dtype=np.float32)
    Wv = np.asarray(Wv, dtype=np.float32)
    Wo = np.asarray(Wo, dtype=np.float32)

    if "nc" not in _NC_CACHE:
        _NC_CACHE["nc"] = _build_nc()
    nc = _NC_CACHE["nc"]

    cos2, sin2p, tri2 = _consts()

    def pack_x(xb):
        # xT [D, T] -> [p][parity][t-half][k2][ti] partition-major
        xT = xb.T.reshape(4, 2, 128, 2, 1024)        # [k2, par, p, half, ti]
        xp = xT.transpose(2, 1, 3, 0, 4).reshape(128, -1)
        return np.ascontiguousarray(xp).astype(NPBF16)

    def pack_w(wT):
        # wT [D, EC] -> [p][k][e]
        wp = wT.reshape(KT, 128, EC).transpose(1, 0, 2).reshape(128, -1)
        return np.ascontiguousarray(wp).astype(NPBF16)

    def pack_wo(woTs):
        # woT slice [EC, D] -> [p][ct][d]
        wp = woTs.reshape(2, 128, D).transpose(1, 0, 2).reshape(128, -1)
        return np.ascontiguousarray(wp).astype(NPBF16)

    xPs = [pack_x(x[b]) for b in range(B)]
    WqT, WkT, WvT, WoT = Wq.T, Wk.T, Wv.T, Wo.T

    in_maps = []
    for c in range(8):
        b, g = c // HG, c % HG
        cs = slice(EC * g, EC * g + EC)
        in_maps.append({
            "xP": xPs[b],
            "wqP": pack_w(WqT[:, cs]),
            "wkP": pack_w(WkT[:, cs]),
            "wvP": pack_w(WvT[:, cs]),
            "woP": pack_wo(WoT[cs, :]),
            "cos2": cos2, "sin2p": sin2p, "tri2": tri2,
        })

    kw = {}
    if _trace:
        kw = dict(trace=True, trace_cores=list(range(8)))
    res = run_bass_kernel_spmd(nc, in_maps, core_ids=list(range(8)), **kw)

    out = np.zeros((B, T, D), np.float32)
    for c in range(8):
        out[c // HG] += res.results[c]["outT"].astype(np.float32).T
    if _trace:
        return out, res
    return out
